# revision 15
# baseline (speedup 1.0000x reference)
"""Paged decoder attention (B=16, S=1, H=16, D=128) on 8 trn2 NeuronCores.

Strategy (tensor-parallel over heads, per sharding hint):
  - Core c owns heads {2c, 2c+1} of every sequence -> 32 (seq, head) tasks
    per core, identical work schedule on every core (pure SPMD).
  - Host gathers each sequence's decode-region KV blocks via block_tables,
    applies the reshape_and_cache update (new token K/V at its slot), and
    lays the data out chunk-major (chunk = 128 contiguous positions):
       K_dev [128=d,   CT]  per chunk: 128 position-columns (d-major)
       V_dev [128=pos, CT]  per chunk: 128 d-columns (pos-major)
  - Device, all PSUM outputs at base partition 0:
      QK:  per chunk, matmul(lhsT=K_tile[d,128pos], rhs=q[d,1])
           -> score columns s_cols[128pos, chunk]
      one PE transpose -> score rows [chunk, 128pos]
      exp (ACT) -> * exp(bias) fused with row-sum (DVE) -> per-chunk sums
      selector matmul collapses chunk sums -> per-task sums -> 1/sum
      one PE transpose of E rows -> E columns [128pos, chunk]
      PV:  per chunk, matmul(lhsT=V_tile[pos,128d], rhs=E_col[pos,1])
           accumulating into o_cols[128d, task]
      PE transpose -> [task, d] rows, scale by 1/sum, DMA out.
  - Softmax max-subtraction is skipped: |scores*scale| <= ~10 here, safely
    inside f32 exp range; exp(bias) is 0 beyond ctx, masking tail+padding.
"""

import numpy as np

B, S, H, D = 16, 1, 16, 128
BLOCK_SIZE = 16
MAX_CTX = 1024
SCALE = 0.08838834764831845
N_CORES = 8
HEADS_PER_CORE = H // N_CORES  # 2
N_TASKS = B * HEADS_PER_CORE   # 32 tasks per core
CHUNK = 128                    # positions per chunk

# dtype of K/V/q/E fed to the tensor engine ("float32" or "bfloat16")
KV_DTYPE = "float32"


def _host_prep(query, key, value, key_cache, value_cache, slot_mapping,
               block_tables, context_lens, attn_bias, max_prompt_len):
    """Gather + layout. Returns per-core input dicts and metadata."""
    q = np.asarray(query, dtype=np.float32).reshape(B, H, D)
    k_new = np.asarray(key, dtype=np.float32).reshape(B, H, D)
    v_new = np.asarray(value, dtype=np.float32).reshape(B, H, D)
    key_cache = np.asarray(key_cache)
    value_cache = np.asarray(value_cache)
    slots = np.asarray(slot_mapping)[:, -1].astype(np.int64)
    bt_full = np.asarray(block_tables)
    ctx = np.asarray(context_lens).astype(np.int64)
    bias = np.asarray(attn_bias, dtype=np.float32)
    kv_dt = np.dtype(np.float32) if KV_DTYPE == "float32" else None

    pt_len = (int(max_prompt_len) + BLOCK_SIZE - 1) // BLOCK_SIZE
    bt = bt_full[:, pt_len:]                      # [B, nb]
    nb = bt.shape[1]
    K_total = nb * BLOCK_SIZE

    ctx = np.clip(ctx, 1, K_total)
    nch = (ctx + CHUNK - 1) // CHUNK              # chunks per sequence [B]

    # chunk table: global chunk id -> (task, local chunk); identical on
    # every core since ctx depends only on b.
    task_b = [t // HEADS_PER_CORE for t in range(N_TASKS)]
    chunk_task = []                               # task id of each chunk
    off = np.zeros(N_TASKS + 1, dtype=np.int64)   # chunk offset per task
    for t in range(N_TASKS):
        off[t + 1] = off[t] + nch[task_b[t]]
        chunk_task += [t] * int(nch[task_b[t]])
    NCH = int(off[-1])
    assert NCH <= 128, f"chunk count {NCH} exceeds one transpose tile"
    CT = NCH * CHUNK

    if KV_DTYPE == "float32":
        to_dev = lambda a: np.ascontiguousarray(a, dtype=np.float32)
        dev_np = np.float32
    else:
        import ml_dtypes
        to_dev = lambda a: np.ascontiguousarray(a).astype(ml_dtypes.bfloat16)
        dev_np = ml_dtypes.bfloat16

    K_dev = [np.zeros((D, CT), dtype=dev_np) for _ in range(N_CORES)]
    V_dev = [np.zeros((D, CT), dtype=dev_np) for _ in range(N_CORES)]
    for b in range(B):
        n = int(ctx[b])
        nblk = (n + BLOCK_SIZE - 1) // BLOCK_SIZE
        blocks = bt[b, :nblk]
        kk = key_cache[blocks]                    # [nblk, H, bs, D]
        kk = np.ascontiguousarray(kk.transpose(1, 0, 2, 3)).reshape(
            H, nblk * BLOCK_SIZE, D)
        vv = value_cache[blocks]                  # [nblk, H, D, bs]
        vv = np.ascontiguousarray(vv.transpose(1, 0, 3, 2)).reshape(
            H, nblk * BLOCK_SIZE, D)

        # reshape_and_cache: place the new token's K/V at its slot position
        sblk = int(slots[b] // BLOCK_SIZE)
        soff = int(slots[b] % BLOCK_SIZE)
        hit = np.nonzero(blocks == sblk)[0]
        if hit.size:
            pos = int(hit[0]) * BLOCK_SIZE + soff
            if pos < nblk * BLOCK_SIZE:
                kk[:, pos, :] = k_new[b]
                vv[:, pos, :] = v_new[b]

        ncb = int(nch[b])
        kpad = np.zeros((H, ncb * CHUNK, D), dtype=np.float32)
        kpad[:, :n] = kk[:, :n]
        vpad = np.zeros((H, ncb * CHUNK, D), dtype=np.float32)
        vpad[:, :n] = vv[:, :n]
        for lh in range(HEADS_PER_CORE):
            t = b * HEADS_PER_CORE + lh
            c0, c1 = int(off[t]) * CHUNK, int(off[t + 1]) * CHUNK
            for c in range(N_CORES):
                h = c * HEADS_PER_CORE + lh
                # K: columns are positions, partition is d
                K_dev[c][:, c0:c1] = to_dev(kpad[h].T)
                # V: per chunk i, 128 d-columns with pos on partitions
                vt = vpad[h].reshape(ncb, CHUNK, D).transpose(1, 0, 2)
                V_dev[c][:, c0:c1] = to_dev(vt.reshape(CHUNK, ncb * D))

    Q_dev = [np.zeros((D, N_TASKS), dtype=dev_np) for _ in range(N_CORES)]
    # exp(bias) rows per (task, chunk): [NCH, 128], 0 beyond ctx
    EB_dev = [np.zeros((NCH, CHUNK), dtype=np.float32) for _ in range(N_CORES)]
    for c in range(N_CORES):
        for t in range(N_TASKS):
            b = task_b[t]
            h = c * HEADS_PER_CORE + (t % HEADS_PER_CORE)
            Q_dev[c][:, t] = to_dev(q[b, h] * SCALE)
            n = int(ctx[b])
            eb = np.zeros(int(nch[b]) * CHUNK, dtype=np.float32)
            eb[:n] = np.exp(bias[b, h, 0, :n].astype(np.float64)).astype(np.float32)
            EB_dev[c][int(off[t]):int(off[t + 1])] = eb.reshape(-1, CHUNK)

    # selector: sel[gc, t] = 1 if chunk gc belongs to task t
    sel = np.zeros((NCH, N_TASKS), dtype=np.float32)
    for gc, t in enumerate(chunk_task):
        sel[gc, t] = 1.0

    # misc blob [128, 192] f32: cols 0:128 eb rows (padded to 128
    # partitions), 128:160 selector, 160:192 q (f32 mode only)
    in_maps = []
    for c in range(N_CORES):
        misc = np.zeros((128, 192), dtype=np.float32)
        misc[:NCH, :CHUNK] = EB_dev[c]
        misc[:NCH, 128:128 + N_TASKS] = sel
        m = {"k_in": K_dev[c], "v_in": V_dev[c], "misc_in": misc}
        if KV_DTYPE == "float32":
            misc[:D, 160:160 + N_TASKS] = Q_dev[c]
        else:
            m["q_in"] = Q_dev[c]
        in_maps.append(m)
    meta = {"NCH": NCH, "CT": CT, "off": off, "chunk_task": chunk_task}
    return in_maps, meta


def _build_program(meta):
    import concourse.bacc as bacc
    import concourse.tile as tile
    from concourse import mybir
    from concourse.masks import make_identity

    NCH, CT = meta["NCH"], meta["CT"]
    off = meta["off"]

    f32 = mybir.dt.float32
    kv_dt = f32 if KV_DTYPE == "float32" else mybir.dt.bfloat16
    nc = bacc.Bacc(None)
    k_in = nc.dram_tensor("k_in", [D, CT], kv_dt, kind="ExternalInput")
    v_in = nc.dram_tensor("v_in", [D, CT], kv_dt, kind="ExternalInput")
    misc_in = nc.dram_tensor("misc_in", [128, 192], f32, kind="ExternalInput")
    if KV_DTYPE != "float32":
        q_in = nc.dram_tensor("q_in", [D, N_TASKS], kv_dt, kind="ExternalInput")
    out = nc.dram_tensor("out", [N_TASKS, D + 1], f32, kind="ExternalOutput")

    SLAB = 8192  # columns per DMA slab

    def slabs(total):
        return [(s, min(SLAB, total - s)) for s in range(0, total, SLAB)]

    with tile.TileContext(nc) as tc:
        with (
            tc.tile_pool(name="kpool", bufs=max(2, len(slabs(CT)))) as kpool,
            tc.tile_pool(name="vpool", bufs=max(2, len(slabs(CT)))) as vpool,
            tc.tile_pool(name="small", bufs=1) as small,
            tc.tile_pool(name="psum", bufs=1, space="PSUM") as psum,
        ):
            ident = small.tile([CHUNK, CHUNK], f32)
            make_identity(nc, ident[:])
            misc_sb = small.tile([128, 192], f32)
            nc.sync.dma_start(out=misc_sb[:], in_=misc_in[:])
            eb_sb = misc_sb[:NCH, :CHUNK]
            sel_sb = misc_sb[:NCH, 128:128 + N_TASKS]
            if KV_DTYPE == "float32":
                q_sb = misc_sb[:D, 160:160 + N_TASKS]
            else:
                q_sb_t = small.tile([D, N_TASKS], kv_dt)
                nc.sync.dma_start(out=q_sb_t[:], in_=q_in[:])
                q_sb = q_sb_t[:]

            # ---- K slabs + QK: score columns ----
            k_tiles = []
            for s, w in slabs(CT):
                kt = kpool.tile([D, SLAB], kv_dt, tag="kslab")
                nc.sync.dma_start(out=kt[:, :w], in_=k_in[:, s:s + w])
                k_tiles.append((s, w, kt))
            # V slabs issued up-front too: DMA engines fill while PE works
            v_tiles = []
            for s, w in slabs(CT):
                vt = vpool.tile([D, SLAB], kv_dt, tag="vslab")
                nc.sync.dma_start(out=vt[:, :w], in_=v_in[:, s:s + w])
                v_tiles.append((s, w, vt))

            # The S3_LW (matmul weight-load) instruction supports a single
            # sync wait; a matmul whose operands arrive on two different DMA
            # semaphore lanes fails codegen. These no-op matmuls absorb one
            # pending wait each so every real matmul needs at most one.
            scratch = psum.tile([1, 1], f32)

            def absorb(ap):
                nc.tensor.matmul(out=scratch[:], lhsT=ap, rhs=ap,
                                 start=True, stop=True)

            absorb(ident[:, :1])
            absorb(misc_sb[:, :1])
            if KV_DTYPE != "float32":
                absorb(q_sb[:, :1])
            # same trick for the DVE: absorb eb's DMA wait early
            dve_scratch = small.tile([1, 1], f32)
            nc.vector.tensor_copy(dve_scratch[:], eb_sb[:1, :1])

            s_cols = psum.tile([CHUNK, NCH], f32)
            for gc in range(NCH):
                t = meta["chunk_task"][gc]
                g = gc * CHUNK
                s, w, kt = k_tiles[g // SLAB]
                nc.tensor.matmul(
                    out=s_cols[:, gc:gc + 1],
                    lhsT=kt[:, g - s:g - s + CHUNK],
                    rhs=q_sb[:, t:t + 1],
                    start=True, stop=True,
                )
            for s, w, vt in v_tiles:
                absorb(vt[:, :1])

            # ---- scores -> rows; exp * exp_bias; row sums ----
            s_cols_sb = small.tile([CHUNK, NCH], f32)
            nc.vector.tensor_copy(s_cols_sb[:], s_cols[:])
            s_rows = psum.tile([NCH, CHUNK], f32)
            nc.tensor.transpose(s_rows[:], s_cols_sb[:], ident[:])
            e_raw = small.tile([NCH, CHUNK], f32)
            nc.scalar.activation(out=e_raw[:], in_=s_rows[:],
                                 func=mybir.ActivationFunctionType.Exp)
            e_sb = small.tile([NCH, CHUNK], kv_dt)
            csums = small.tile([NCH, 1], f32)
            nc.vector.tensor_tensor(out=e_sb[:], in0=e_raw[:], in1=eb_sb,
                                    op=mybir.AluOpType.mult)
            nc.vector.reduce_sum(out=csums[:], in_=e_sb[:],
                                 axis=mybir.AxisListType.X)

            # ---- per-task sums (normalization happens on host) ----
            tsums = psum.tile([N_TASKS, 1], f32)
            nc.tensor.matmul(out=tsums[:], lhsT=sel_sb, rhs=csums[:],
                             start=True, stop=True)

            # ---- E rows -> E columns ----
            e_cols_ps = psum.tile([CHUNK, NCH], f32)
            nc.tensor.transpose(e_cols_ps[:], e_sb[:], ident[:NCH, :NCH])
            e_cols = small.tile([CHUNK, NCH], kv_dt)
            nc.vector.tensor_copy(e_cols[:], e_cols_ps[:])

            # ---- PV: accumulate output columns ----
            o_cols = psum.tile([D, N_TASKS], f32)
            for t in range(N_TASKS):
                t0, t1 = int(off[t]), int(off[t + 1])
                for gc in range(t0, t1):
                    g = gc * CHUNK
                    s, w, vt = v_tiles[g // SLAB]
                    nc.tensor.matmul(
                        out=o_cols[:, t:t + 1],
                        lhsT=vt[:, g - s:g - s + CHUNK],
                        rhs=e_cols[:, gc:gc + 1],
                        start=(gc == t0), stop=(gc == t1 - 1),
                    )

            # ---- transpose to rows, store (unnormalized) + sums ----
            o_sb = small.tile([D, N_TASKS], f32)
            nc.vector.tensor_copy(o_sb[:], o_cols[:])
            o_rows = psum.tile([N_TASKS, D], f32)
            nc.tensor.transpose(o_rows[:], o_sb[:], ident[:])
            out_sb = small.tile([N_TASKS, D + 1], f32)
            nc.vector.tensor_copy(out_sb[:, :D], o_rows[:])
            nc.vector.tensor_copy(out_sb[:, D:D + 1], tsums[:])
            nc.sync.dma_start(out=out[:], in_=out_sb[:])

    nc.compile()
    return nc


LAST_RESULT = None  # BassKernelResults from the most recent run (for profiling)


def kernel(**inputs) -> np.ndarray:
    import os
    from concourse.bass_utils import run_bass_kernel_spmd

    in_maps, meta = _host_prep(**inputs)
    nc = _build_program(meta)
    trace = bool(os.environ.get("KERNEL_TRACE"))
    res = run_bass_kernel_spmd(nc, in_maps, core_ids=list(range(N_CORES)),
                               trace=trace)
    global LAST_RESULT
    LAST_RESULT = res

    out = np.zeros((B, S, H * D), dtype=np.float32)
    for c in range(N_CORES):
        blob = res.results[c]["out"]              # [32, 129] rows|sum
        rows = blob[:, :D] / blob[:, D:D + 1]
        for t in range(N_TASKS):
            b = t // HEADS_PER_CORE
            h = c * HEADS_PER_CORE + (t % HEADS_PER_CORE)
            out[b, 0, h * D:(h + 1) * D] = rows[t]
    return out


# revision 17
# speedup vs baseline: 3.4912x; 3.4912x over previous
"""Paged decoder attention (B=16, S=1, H=16, D=128) on 8 trn2 NeuronCores.

Strategy (tensor-parallel over heads, per sharding hint):
  - Core c owns heads {2c, 2c+1} of every sequence -> 32 (seq, head) tasks
    per core, identical work schedule on every core (pure SPMD).
  - Host gathers each sequence's decode-region KV blocks via block_tables,
    applies the reshape_and_cache update (new token K/V at its slot), and
    lays the data out chunk-major (chunk = 128 contiguous positions):
       K_dev [128=d,   CT]  per chunk: 128 position-columns (d-major)
       V_dev [128=pos, CT]  per chunk: 128 d-columns (pos-major)
  - Device, all PSUM outputs at base partition 0:
      QK:  per chunk, matmul(lhsT=K_tile[d,128pos], rhs=q[d,1])
           -> score columns s_cols[128pos, chunk]
      one PE transpose -> score rows [chunk, 128pos]
      exp (ACT) -> * exp(bias) fused with row-sum (DVE) -> per-chunk sums
      selector matmul collapses chunk sums -> per-task sums -> 1/sum
      one PE transpose of E rows -> E columns [128pos, chunk]
      PV:  per chunk, matmul(lhsT=V_tile[pos,128d], rhs=E_col[pos,1])
           accumulating into o_cols[128d, task]
      PE transpose -> [task, d] rows, scale by 1/sum, DMA out.
  - Softmax max-subtraction is skipped: |scores*scale| <= ~10 here, safely
    inside f32 exp range; exp(bias) is 0 beyond ctx, masking tail+padding.
"""

import numpy as np

B, S, H, D = 16, 1, 16, 128
BLOCK_SIZE = 16
MAX_CTX = 1024
SCALE = 0.08838834764831845
N_CORES = 8
HEADS_PER_CORE = H // N_CORES  # 2
N_TASKS = B * HEADS_PER_CORE   # 32 tasks per core
CHUNK = 128                    # positions per chunk

# dtype of K/V/q/E fed to the tensor engine ("float32" or "bfloat16")
KV_DTYPE = "bfloat16"


def _host_prep(query, key, value, key_cache, value_cache, slot_mapping,
               block_tables, context_lens, attn_bias, max_prompt_len):
    """Gather + layout. Returns per-core input dicts and metadata."""
    q = np.asarray(query, dtype=np.float32).reshape(B, H, D)
    k_new = np.asarray(key, dtype=np.float32).reshape(B, H, D)
    v_new = np.asarray(value, dtype=np.float32).reshape(B, H, D)
    key_cache = np.asarray(key_cache)
    value_cache = np.asarray(value_cache)
    slots = np.asarray(slot_mapping)[:, -1].astype(np.int64)
    bt_full = np.asarray(block_tables)
    ctx = np.asarray(context_lens).astype(np.int64)
    bias = np.asarray(attn_bias, dtype=np.float32)
    kv_dt = np.dtype(np.float32) if KV_DTYPE == "float32" else None

    pt_len = (int(max_prompt_len) + BLOCK_SIZE - 1) // BLOCK_SIZE
    bt = bt_full[:, pt_len:]                      # [B, nb]
    nb = bt.shape[1]
    K_total = nb * BLOCK_SIZE

    ctx = np.clip(ctx, 1, K_total)
    nch = (ctx + CHUNK - 1) // CHUNK              # chunks per sequence [B]

    # chunk table: global chunk id -> (task, local chunk); identical on
    # every core since ctx depends only on b.
    task_b = [t // HEADS_PER_CORE for t in range(N_TASKS)]
    chunk_task = []                               # task id of each chunk
    off = np.zeros(N_TASKS + 1, dtype=np.int64)   # chunk offset per task
    for t in range(N_TASKS):
        off[t + 1] = off[t] + nch[task_b[t]]
        chunk_task += [t] * int(nch[task_b[t]])
    NCH = int(off[-1])
    assert NCH <= 128, f"chunk count {NCH} exceeds one transpose tile"
    CT = NCH * CHUNK

    if KV_DTYPE == "float32":
        to_dev = lambda a: np.ascontiguousarray(a, dtype=np.float32)
        dev_np = np.float32
    else:
        import ml_dtypes
        to_dev = lambda a: np.ascontiguousarray(a).astype(ml_dtypes.bfloat16)
        dev_np = ml_dtypes.bfloat16

    K_dev = [np.zeros((D, CT), dtype=dev_np) for _ in range(N_CORES)]
    V_dev = [np.zeros((D, CT), dtype=dev_np) for _ in range(N_CORES)]
    for b in range(B):
        n = int(ctx[b])
        nblk = (n + BLOCK_SIZE - 1) // BLOCK_SIZE
        blocks = bt[b, :nblk]
        kk = key_cache[blocks]                    # [nblk, H, bs, D]
        kk = np.ascontiguousarray(kk.transpose(1, 0, 2, 3)).reshape(
            H, nblk * BLOCK_SIZE, D)
        vv = value_cache[blocks]                  # [nblk, H, D, bs]
        vv = np.ascontiguousarray(vv.transpose(1, 0, 3, 2)).reshape(
            H, nblk * BLOCK_SIZE, D)

        # reshape_and_cache: place the new token's K/V at its slot position
        sblk = int(slots[b] // BLOCK_SIZE)
        soff = int(slots[b] % BLOCK_SIZE)
        hit = np.nonzero(blocks == sblk)[0]
        if hit.size:
            pos = int(hit[0]) * BLOCK_SIZE + soff
            if pos < nblk * BLOCK_SIZE:
                kk[:, pos, :] = k_new[b]
                vv[:, pos, :] = v_new[b]

        ncb = int(nch[b])
        kpad = np.zeros((H, ncb * CHUNK, D), dtype=np.float32)
        kpad[:, :n] = kk[:, :n]
        vpad = np.zeros((H, ncb * CHUNK, D), dtype=np.float32)
        vpad[:, :n] = vv[:, :n]
        for lh in range(HEADS_PER_CORE):
            t = b * HEADS_PER_CORE + lh
            c0, c1 = int(off[t]) * CHUNK, int(off[t + 1]) * CHUNK
            for c in range(N_CORES):
                h = c * HEADS_PER_CORE + lh
                # K: columns are positions, partition is d
                K_dev[c][:, c0:c1] = to_dev(kpad[h].T)
                # V: per chunk i, 128 d-columns with pos on partitions
                vt = vpad[h].reshape(ncb, CHUNK, D).transpose(1, 0, 2)
                V_dev[c][:, c0:c1] = to_dev(vt.reshape(CHUNK, ncb * D))

    Q_dev = [np.zeros((D, N_TASKS), dtype=dev_np) for _ in range(N_CORES)]
    # exp(bias) rows per (task, chunk): [NCH, 128], 0 beyond ctx
    EB_dev = [np.zeros((NCH, CHUNK), dtype=np.float32) for _ in range(N_CORES)]
    for c in range(N_CORES):
        for t in range(N_TASKS):
            b = task_b[t]
            h = c * HEADS_PER_CORE + (t % HEADS_PER_CORE)
            Q_dev[c][:, t] = to_dev(q[b, h] * SCALE)
            n = int(ctx[b])
            eb = np.zeros(int(nch[b]) * CHUNK, dtype=np.float32)
            eb[:n] = np.exp(bias[b, h, 0, :n].astype(np.float64)).astype(np.float32)
            EB_dev[c][int(off[t]):int(off[t + 1])] = eb.reshape(-1, CHUNK)

    # selector: sel[gc, t] = 1 if chunk gc belongs to task t
    sel = np.zeros((NCH, N_TASKS), dtype=np.float32)
    for gc, t in enumerate(chunk_task):
        sel[gc, t] = 1.0

    # misc blob [128, 192] f32: cols 0:128 eb rows (padded to 128
    # partitions), 128:160 selector, 160:192 q (f32 mode only)
    in_maps = []
    for c in range(N_CORES):
        misc = np.zeros((128, 192), dtype=np.float32)
        misc[:NCH, :CHUNK] = EB_dev[c]
        misc[:NCH, 128:128 + N_TASKS] = sel
        m = {"k_in": K_dev[c], "v_in": V_dev[c], "misc_in": misc}
        if KV_DTYPE == "float32":
            misc[:D, 160:160 + N_TASKS] = Q_dev[c]
        else:
            m["q_in"] = Q_dev[c]
        in_maps.append(m)
    meta = {"NCH": NCH, "CT": CT, "off": off, "chunk_task": chunk_task}
    return in_maps, meta


def _build_program(meta):
    import concourse.bacc as bacc
    import concourse.tile as tile
    from concourse import mybir
    from concourse.masks import make_identity

    NCH, CT = meta["NCH"], meta["CT"]
    off = meta["off"]

    f32 = mybir.dt.float32
    kv_dt = f32 if KV_DTYPE == "float32" else mybir.dt.bfloat16
    nc = bacc.Bacc(None)
    k_in = nc.dram_tensor("k_in", [D, CT], kv_dt, kind="ExternalInput")
    v_in = nc.dram_tensor("v_in", [D, CT], kv_dt, kind="ExternalInput")
    misc_in = nc.dram_tensor("misc_in", [128, 192], f32, kind="ExternalInput")
    if KV_DTYPE != "float32":
        q_in = nc.dram_tensor("q_in", [D, N_TASKS], kv_dt, kind="ExternalInput")
    out = nc.dram_tensor("out", [N_TASKS, D + 1], f32, kind="ExternalOutput")

    SLAB = 8192  # columns per DMA slab

    def slabs(total):
        return [(s, min(SLAB, total - s)) for s in range(0, total, SLAB)]

    with tile.TileContext(nc) as tc:
        with (
            tc.tile_pool(name="kpool", bufs=max(2, len(slabs(CT)))) as kpool,
            tc.tile_pool(name="vpool", bufs=max(2, len(slabs(CT)))) as vpool,
            tc.tile_pool(name="small", bufs=1) as small,
            tc.tile_pool(name="psum", bufs=1, space="PSUM") as psum,
        ):
            ident = small.tile([CHUNK, CHUNK], f32)
            make_identity(nc, ident[:])
            misc_sb = small.tile([128, 192], f32)
            nc.sync.dma_start(out=misc_sb[:], in_=misc_in[:])
            eb_sb = misc_sb[:NCH, :CHUNK]
            sel_sb = misc_sb[:NCH, 128:128 + N_TASKS]
            if KV_DTYPE == "float32":
                q_sb = misc_sb[:D, 160:160 + N_TASKS]
            else:
                q_sb_t = small.tile([D, N_TASKS], kv_dt)
                nc.sync.dma_start(out=q_sb_t[:], in_=q_in[:])
                q_sb = q_sb_t[:]

            # ---- K slabs + QK: score columns ----
            k_tiles = []
            for s, w in slabs(CT):
                kt = kpool.tile([D, SLAB], kv_dt, tag="kslab")
                nc.sync.dma_start(out=kt[:, :w], in_=k_in[:, s:s + w])
                k_tiles.append((s, w, kt))
            # V slabs issued up-front too: DMA engines fill while PE works
            v_tiles = []
            for s, w in slabs(CT):
                vt = vpool.tile([D, SLAB], kv_dt, tag="vslab")
                nc.sync.dma_start(out=vt[:, :w], in_=v_in[:, s:s + w])
                v_tiles.append((s, w, vt))

            # The S3_LW (matmul weight-load) instruction supports a single
            # sync wait; a matmul whose operands arrive on two different DMA
            # semaphore lanes fails codegen. These no-op matmuls absorb one
            # pending wait each so every real matmul needs at most one.
            scratch = psum.tile([1, 1], f32)

            def absorb(ap):
                nc.tensor.matmul(out=scratch[:], lhsT=ap, rhs=ap,
                                 start=True, stop=True)

            absorb(ident[:, :1])
            absorb(misc_sb[:, :1])
            if KV_DTYPE != "float32":
                absorb(q_sb[:, :1])
            # same trick for the DVE: absorb eb's DMA wait early
            dve_scratch = small.tile([1, 1], f32)
            nc.vector.tensor_copy(dve_scratch[:], eb_sb[:1, :1])

            s_cols = psum.tile([CHUNK, NCH], f32)
            for gc in range(NCH):
                t = meta["chunk_task"][gc]
                g = gc * CHUNK
                s, w, kt = k_tiles[g // SLAB]
                nc.tensor.matmul(
                    out=s_cols[:, gc:gc + 1],
                    lhsT=kt[:, g - s:g - s + CHUNK],
                    rhs=q_sb[:, t:t + 1],
                    start=True, stop=True,
                )
            for s, w, vt in v_tiles:
                absorb(vt[:, :1])

            # ---- scores -> rows; exp * exp_bias; row sums ----
            s_cols_sb = small.tile([CHUNK, NCH], f32)
            nc.vector.tensor_copy(s_cols_sb[:], s_cols[:])
            s_rows = psum.tile([NCH, CHUNK], f32)
            nc.tensor.transpose(s_rows[:], s_cols_sb[:], ident[:])
            e_raw = small.tile([NCH, CHUNK], f32)
            nc.scalar.activation(out=e_raw[:], in_=s_rows[:],
                                 func=mybir.ActivationFunctionType.Exp)
            e_sb = small.tile([NCH, CHUNK], f32)
            csums = small.tile([NCH, 1], f32)
            nc.vector.tensor_tensor(out=e_sb[:], in0=e_raw[:], in1=eb_sb,
                                    op=mybir.AluOpType.mult)
            nc.vector.reduce_sum(out=csums[:], in_=e_sb[:],
                                 axis=mybir.AxisListType.X)

            # ---- per-task sums (normalization happens on host) ----
            tsums = psum.tile([N_TASKS, 1], f32)
            nc.tensor.matmul(out=tsums[:], lhsT=sel_sb, rhs=csums[:],
                             start=True, stop=True)

            # ---- E rows -> E columns ----
            e_cols_ps = psum.tile([CHUNK, NCH], f32)
            nc.tensor.transpose(e_cols_ps[:], e_sb[:], ident[:NCH, :NCH])
            e_cols = small.tile([CHUNK, NCH], kv_dt)
            nc.vector.tensor_copy(e_cols[:], e_cols_ps[:])

            # ---- PV: accumulate output columns ----
            o_cols = psum.tile([D, N_TASKS], f32)
            for t in range(N_TASKS):
                t0, t1 = int(off[t]), int(off[t + 1])
                for gc in range(t0, t1):
                    g = gc * CHUNK
                    s, w, vt = v_tiles[g // SLAB]
                    nc.tensor.matmul(
                        out=o_cols[:, t:t + 1],
                        lhsT=vt[:, g - s:g - s + CHUNK],
                        rhs=e_cols[:, gc:gc + 1],
                        start=(gc == t0), stop=(gc == t1 - 1),
                    )

            # ---- transpose to rows, store (unnormalized) + sums ----
            o_sb = small.tile([D, N_TASKS], f32)
            nc.vector.tensor_copy(o_sb[:], o_cols[:])
            o_rows = psum.tile([N_TASKS, D], f32)
            nc.tensor.transpose(o_rows[:], o_sb[:], ident[:])
            out_sb = small.tile([N_TASKS, D + 1], f32)
            nc.vector.tensor_copy(out_sb[:, :D], o_rows[:])
            nc.vector.tensor_copy(out_sb[:, D:D + 1], tsums[:])
            nc.sync.dma_start(out=out[:], in_=out_sb[:])

    nc.compile()
    return nc


LAST_RESULT = None  # BassKernelResults from the most recent run (for profiling)


def kernel(**inputs) -> np.ndarray:
    import os
    from concourse.bass_utils import run_bass_kernel_spmd

    in_maps, meta = _host_prep(**inputs)
    nc = _build_program(meta)
    trace = bool(os.environ.get("KERNEL_TRACE"))
    res = run_bass_kernel_spmd(nc, in_maps, core_ids=list(range(N_CORES)),
                               trace=trace)
    global LAST_RESULT
    LAST_RESULT = res

    out = np.zeros((B, S, H * D), dtype=np.float32)
    for c in range(N_CORES):
        blob = res.results[c]["out"]              # [32, 129] rows|sum
        rows = blob[:, :D] / blob[:, D:D + 1]
        for t in range(N_TASKS):
            b = t // HEADS_PER_CORE
            h = c * HEADS_PER_CORE + (t % HEADS_PER_CORE)
            out[b, 0, h * D:(h + 1) * D] = rows[t]
    return out
